# revision 3
# baseline (speedup 1.0000x reference)
"""Expert-parallel MoE grouped-experts kernel for 8 trn2 NeuronCores.

Contract: kernel(**inputs) takes FULL unsharded inputs, returns FULL output.

Strategy (expert-parallel):
  - Host: sort token-expert assignments by expert, gather each core's tokens
    (cores own 4 experts each), pad per-expert blocks to a common capacity.
  - Device (SPMD x8): grouped GEMMs per expert: g = x@gwT, u = x@uwT,
    hmid = silu(g)*u, o = hmid@dwT. fp32 PSUM accumulation everywhere.
  - Host: scale by routing weights, scatter-add back to token order.

Two matmul dtype variants:
  "bf16": weights fully resident per expert; rows stream in 512-slices.
  "tf32" (float32r): tf32 precision at full PE speed. Bigger footprint, so
     activations+hmid stay resident in 1152-row chunks while weight i-tiles
     and h-slices stream through.
"""
import os
import sys

if "/opt/trn_rl_repo" not in sys.path:
    sys.path.insert(0, "/opt/trn_rl_repo")

import math
import numpy as np
import ml_dtypes

B, S, H, I, E, K = 4, 4096, 2048, 1024, 32, 4
N = B * S
NCORES = 8
EPC = E // NCORES  # experts per core
HC = H // 128      # 16 h-chunks
IC = I // 128      # 8 i-chunks
IT = I // 128      # 8 i-tiles of 128

VARIANT = os.environ.get("MOE_VARIANT", "tf32")

_LAST_RESULTS = None  # BassKernelResults of the most recent run (for test.py)


def _round_tf32(x: np.ndarray) -> np.ndarray:
    """Round f32 to tf32 (10-bit mantissa), round-to-nearest-even."""
    u = np.ascontiguousarray(x, dtype=np.float32).view(np.uint32).astype(np.uint64)
    u = u + 0x0FFF + ((u >> 13) & 1)
    u = (u & np.uint64(0xFFFFE000)).astype(np.uint32)
    return u.view(np.float32)


def _row_chunks(ce: int, chunk: int = 1152):
    """Split an expert block of ce rows into resident chunks."""
    out = []
    r = 0
    while r < ce:
        out.append((r, min(chunk, ce - r)))
        r += out[-1][1]
    return out


def _move_slices(length: int):
    """Split a chunk into moving-dim slices (prefer >=256 for f32r speed)."""
    out = []
    r = 0
    while r < length:
        rem = length - r
        if rem <= 512:
            s = rem
        elif rem - 384 >= 256:
            s = 384
        else:
            s = 512
        out.append((r, s))
        r += s
    return out


def _build_bf16(CT: int):
    import concourse.tile as tile
    import concourse.mybir as mybir
    from concourse import bacc

    bf16 = mybir.dt.bfloat16
    f32 = mybir.dt.float32

    nc = bacc.Bacc("TRN2", target_bir_lowering=False, debug=False)

    xsT = nc.dram_tensor("xsT", [H, CT], bf16, kind="ExternalInput")
    gwT = nc.dram_tensor("gwT", [EPC, H, I], bf16, kind="ExternalInput")
    uwT = nc.dram_tensor("uwT", [EPC, H, I], bf16, kind="ExternalInput")
    dwT = nc.dram_tensor("dwT", [EPC, I, H], bf16, kind="ExternalInput")
    o = nc.dram_tensor("o", [CT, H], f32, kind="ExternalOutput")

    Ce = CT // EPC
    slices = []
    r = 0
    while r < Ce:
        rl = min(512, Ce - r)
        slices.append((r, rl))
        r += rl

    with tile.TileContext(nc) as tc:
        with (
            tc.tile_pool(name="wg", bufs=1) as wg,
            tc.tile_pool(name="wu", bufs=1) as wu,
            tc.tile_pool(name="wd", bufs=1) as wd,
            tc.tile_pool(name="xs", bufs=2) as xsp,
            tc.tile_pool(name="hm", bufs=2) as hmp,
            tc.tile_pool(name="sg", bufs=2) as sgp,
            tc.tile_pool(name="ost", bufs=4) as ostp,
            tc.tile_pool(name="psum", bufs=6, space="PSUM") as psp,
        ):
            for e in range(EPC):
                base = e * Ce
                gw = wg.tile([128, HC, I], bf16, tag="gw")
                nc.sync.dma_start(gw[:], gwT.ap()[e].rearrange("(c p) i -> p c i", p=128))
                uw = wu.tile([128, HC, I], bf16, tag="uw")
                nc.sync.dma_start(uw[:], uwT.ap()[e].rearrange("(c p) i -> p c i", p=128))
                dw = wd.tile([128, IC, H], bf16, tag="dw")
                nc.sync.dma_start(dw[:], dwT.ap()[e].rearrange("(c p) h -> p c h", p=128))

                for r0, rl in slices:
                    xst = xsp.tile([128, HC, rl], bf16, tag="xs")
                    nc.sync.dma_start(
                        xst[:],
                        xsT.ap()[:, base + r0 : base + r0 + rl].rearrange(
                            "(c p) r -> p c r", p=128
                        ),
                    )
                    hm = hmp.tile([128, IC, rl], bf16, tag="hm")
                    for it in range(IT):
                        pg = psp.tile([128, rl], mybir.dt.float32, tag="ps")
                        pu = psp.tile([128, rl], mybir.dt.float32, tag="ps")
                        for hc in range(HC):
                            nc.tensor.matmul(
                                pg[:], gw[:, hc, it * 128 : (it + 1) * 128],
                                xst[:, hc, :], start=(hc == 0), stop=(hc == HC - 1),
                            )
                        for hc in range(HC):
                            nc.tensor.matmul(
                                pu[:], uw[:, hc, it * 128 : (it + 1) * 128],
                                xst[:, hc, :], start=(hc == 0), stop=(hc == HC - 1),
                            )
                        sg = sgp.tile([128, rl], mybir.dt.float32, tag="sg")
                        nc.scalar.activation(sg[:], pg[:], mybir.ActivationFunctionType.Silu)
                        nc.vector.tensor_mul(hm[:, it, :], sg[:], pu[:])

                    for rt in range(rl // 128):
                        for hs in range(H // 512):
                            po = psp.tile([128, 512], mybir.dt.float32, tag="ps")
                            for ic in range(IC):
                                nc.tensor.matmul(
                                    po[:], hm[:, ic, rt * 128 : (rt + 1) * 128],
                                    dw[:, ic, hs * 512 : (hs + 1) * 512],
                                    start=(ic == 0), stop=(ic == IC - 1),
                                )
                            ot = ostp.tile([128, 512], mybir.dt.float32, tag="o")
                            nc.vector.tensor_copy(ot[:], po[:])
                            nc.sync.dma_start(
                                o.ap()[
                                    base + r0 + rt * 128 : base + r0 + (rt + 1) * 128,
                                    hs * 512 : (hs + 1) * 512,
                                ],
                                ot[:],
                            )
    nc.compile()
    return nc


def _build_tf32(CT: int):
    """f32r everywhere. Activations (xs chunk) + hmid resident; weights stream.

    guw DRAM layout (host-prepared): [EPC, IT, 128p, HC, 128i] so one i-tile
    stationary block loads as 128 contiguous 8KB lines.
    """
    import concourse.tile as tile
    import concourse.mybir as mybir
    from concourse import bacc

    f32 = mybir.dt.float32
    f32r = mybir.dt.float32r

    nc = bacc.Bacc("TRN2", target_bir_lowering=False, debug=False)

    xsT = nc.dram_tensor("xsT", [H, CT], f32r, kind="ExternalInput")
    gwP = nc.dram_tensor("gwP", [EPC, IT, 128, HC, 128], f32r, kind="ExternalInput")
    uwP = nc.dram_tensor("uwP", [EPC, IT, 128, HC, 128], f32r, kind="ExternalInput")
    dwT = nc.dram_tensor("dwT", [EPC, I, H], f32r, kind="ExternalInput")
    o = nc.dram_tensor("o", [CT, H], f32, kind="ExternalOutput")

    Ce = CT // EPC
    chunks = _row_chunks(Ce)

    with tile.TileContext(nc) as tc:
        with (
            tc.tile_pool(name="xs", bufs=1) as xsp,
            tc.tile_pool(name="wg", bufs=2) as wg,
            tc.tile_pool(name="wu", bufs=2) as wu,
            tc.tile_pool(name="wd", bufs=2) as wd,
            tc.tile_pool(name="hm", bufs=1) as hmp,
            tc.tile_pool(name="sg", bufs=2) as sgp,
            tc.tile_pool(name="ost", bufs=4) as ostp,
            tc.tile_pool(name="psum", bufs=6, space="PSUM") as psp,
        ):
            for e in range(EPC):
                for c0, cl in chunks:
                    base = e * Ce + c0
                    xst_full = xsp.tile([128, HC, 1152], f32r, tag="xs")
                    xst = xst_full[:, :, :cl]
                    nc.sync.dma_start(
                        xst[:],
                        xsT.ap()[:, base : base + cl].rearrange("(c p) r -> p c r", p=128),
                    )
                    hm_full = hmp.tile([128, IC, 1152], f32r, tag="hm")
                    hm = hm_full[:, :, :cl]
                    for it in range(IT):
                        gw = wg.tile([128, HC, 128], f32r, tag="gw")
                        nc.sync.dma_start(gw[:], gwP.ap()[e, it])
                        uw = wu.tile([128, HC, 128], f32r, tag="uw")
                        nc.sync.dma_start(uw[:], uwP.ap()[e, it])
                        for r0, rl in _move_slices(cl):
                            pg_full = psp.tile([128, 512], f32, tag="ps")
                            pu_full = psp.tile([128, 512], f32, tag="ps")
                            pg = pg_full[:, :rl]
                            pu = pu_full[:, :rl]
                            for hc in range(HC):
                                nc.tensor.matmul(
                                    pg[:], gw[:, hc, :], xst[:, hc, r0 : r0 + rl],
                                    start=(hc == 0), stop=(hc == HC - 1),
                                )
                            for hc in range(HC):
                                nc.tensor.matmul(
                                    pu[:], uw[:, hc, :], xst[:, hc, r0 : r0 + rl],
                                    start=(hc == 0), stop=(hc == HC - 1),
                                )
                            sg_full = sgp.tile([128, 512], f32, tag="sg")
                            sg = sg_full[:, :rl]
                            nc.scalar.activation(
                                sg[:], pg[:], mybir.ActivationFunctionType.Silu
                            )
                            nc.vector.tensor_mul(hm[:, it, r0 : r0 + rl], sg[:], pu[:])

                    for hs in range(H // 512):
                        dw = wd.tile([128, IC, 512], f32r, tag="dw")
                        nc.sync.dma_start(
                            dw[:],
                            dwT.ap()[e][:, hs * 512 : (hs + 1) * 512].rearrange(
                                "(c p) h -> p c h", p=128
                            ),
                        )
                        for rt in range(cl // 128):
                            po = psp.tile([128, 512], f32, tag="ps")
                            for ic in range(IC):
                                nc.tensor.matmul(
                                    po[:], hm[:, ic, rt * 128 : (rt + 1) * 128],
                                    dw[:, ic, :], start=(ic == 0), stop=(ic == IC - 1),
                                )
                            ot = ostp.tile([128, 512], f32, tag="o")
                            nc.vector.tensor_copy(ot[:], po[:])
                            nc.sync.dma_start(
                                o.ap()[
                                    base + rt * 128 : base + (rt + 1) * 128,
                                    hs * 512 : (hs + 1) * 512,
                                ],
                                ot[:],
                            )
    nc.compile()
    return nc


def kernel(hidden_states, gate_weight, up_weight, down_weight, topk_idx, topk_weight):
    global _LAST_RESULTS
    from concourse.bass_utils import run_bass_kernel_spmd

    bf16 = ml_dtypes.bfloat16
    variant = VARIANT

    x = np.ascontiguousarray(hidden_states, dtype=np.float32).reshape(N, H)
    flat_expert = np.asarray(topk_idx).reshape(-1).astype(np.int64)
    flat_weight = np.asarray(topk_weight).reshape(-1).astype(np.float32)

    perm = np.argsort(flat_expert, kind="stable")
    tok_sorted = np.repeat(np.arange(N), K)[perm]
    sizes = np.bincount(flat_expert, minlength=E)
    offs = np.concatenate([[0], np.cumsum(sizes)])

    Ce = int(math.ceil(sizes.max() / 256) * 256)
    CT = EPC * Ce

    gw_all = np.asarray(gate_weight, dtype=np.float32)
    uw_all = np.asarray(up_weight, dtype=np.float32)
    dw_all = np.asarray(down_weight, dtype=np.float32)

    in_maps = []
    for m in range(NCORES):
        sl = slice(m * EPC, (m + 1) * EPC)
        if variant == "bf16":
            xsT_m = np.zeros((H, CT), dtype=bf16)
            for el in range(EPC):
                ex = m * EPC + el
                ids = tok_sorted[offs[ex] : offs[ex + 1]]
                xsT_m[:, el * Ce : el * Ce + len(ids)] = x[ids].astype(bf16).T
            in_maps.append(
                {
                    "xsT": xsT_m,
                    "gwT": np.ascontiguousarray(gw_all[sl].transpose(0, 2, 1)).astype(bf16),
                    "uwT": np.ascontiguousarray(uw_all[sl].transpose(0, 2, 1)).astype(bf16),
                    "dwT": np.ascontiguousarray(dw_all[sl].transpose(0, 2, 1)).astype(bf16),
                }
            )
        else:
            xsT_m = np.zeros((H, CT), dtype=np.float32)
            for el in range(EPC):
                ex = m * EPC + el
                ids = tok_sorted[offs[ex] : offs[ex + 1]]
                xsT_m[:, el * Ce : el * Ce + len(ids)] = _round_tf32(x[ids]).T
            # gwP[e, it, p, hc, il] = gate_weight[i, h] with i=it*128+il, h=hc*128+p
            def pack_gu(w):  # w: (EPC, I, H)
                w4 = w.reshape(EPC, IT, 128, HC, 128)  # (e, it, il, hc, p)
                return _round_tf32(np.ascontiguousarray(w4.transpose(0, 1, 4, 3, 2)))

            in_maps.append(
                {
                    "xsT": xsT_m,
                    "gwP": pack_gu(gw_all[sl]),
                    "uwP": pack_gu(uw_all[sl]),
                    "dwT": _round_tf32(
                        np.ascontiguousarray(dw_all[sl].transpose(0, 2, 1))
                    ),
                }
            )

    nc = _build_bf16(CT) if variant == "bf16" else _build_tf32(CT)
    res = run_bass_kernel_spmd(nc, in_maps, core_ids=list(range(NCORES)))
    _LAST_RESULTS = res

    # combine: weighted scatter-add back to token order
    o_sorted = np.empty((N * K, H), dtype=np.float32)
    for m in range(NCORES):
        om = res.results[m]["o"]
        for el in range(EPC):
            ex = m * EPC + el
            n_e = offs[ex + 1] - offs[ex]
            o_sorted[offs[ex] : offs[ex + 1]] = om[el * Ce : el * Ce + n_e]
    o_sorted *= flat_weight[perm][:, None]
    o_orig = np.empty_like(o_sorted)
    o_orig[perm] = o_sorted
    y = o_orig.reshape(N, K, H).sum(axis=1)
    return y.reshape(B, S, H).astype(np.float32)
